# revision 30
# baseline (speedup 1.0000x reference)
"""GNN message-passing (SAGE-gcn + GraphConv stack) Trainium2 Bass kernel.

Strategy (8 NeuronCores, dst-node sharded):
- Each core owns a contiguous block of N/8 dst nodes and all edges into them.
- Per layer, the small weight matmul is applied BEFORE aggregation
  (A @ (h W^T) == (A @ h) W^T), shrinking the gathered message dim.
- z tables are stored in float8e4 with per-layer power-of-2 scales (folded
  into host-provided cast/epilogue scale tables); rows padded to 256B
  multiples (dma_gather constraint). Halves gather DMA bytes on the wide
  layers and enables DoubleRow fp8 matmuls (2 chunks per instruction).
- Each core computes z for its own nodes; an in-kernel AllGather replicates
  the full z table (fp8) into every core's DRAM. Each AG slab carries a
  dedicated all-zero row (row 0 of half 0; the first pad-node row of half 1)
  used as the gather target for empty identity slots.
- Aggregation, identity-chunk layout: per (half, dst-tile) cell, edges whose
  src has a single edge into the cell are placed at gather position = their
  dst slot, Q=12 levels deep -> the one-hot S for those chunks is a CONSTANT
  identity (no S traffic at all). Multi-edge srcs (dedup pairs riding a
  merged 0/1/2 column) and singles deeper than Q go to a small "overflow"
  region with host-precomputed fp8 S matrices, small enough to PIN in SBUF
  across all 6 layers. Consecutive chunk pairs run as one DoubleRow matmul.
- Self-loops are removed from the gather stream. The sage "+h" term and the
  self-loop contribution ride a per-tile "local chunk" (contiguous HWDGE load
  of the core's own z rows) multiplied by 2I (sage) or I (gc).
- Host does index-only preprocessing: edge bucketing, identity-slot layout,
  int16 gather indices (two ~25k-row half tables), degree scalars, S tables.
- The two gather tables are node-range halves (n < 3136 within the shard) so
  table 0's AllGather fires mid-layer (after tile 24's stage_b) and overlaps
  compute; only table 1's AllGather sits at the layer end, and strip 0 of the
  next layer runs phase-A (table-0 jobs only, partial spilled to SBUF bf16)
  to absorb its latency.
- Layer 0 gathers raw fp8 features from host-replicated half tables (same
  layout the AG would produce) and applies W0 after aggregation, removing
  stage B0's AllGathers from the critical path; a tiny warmup AllGather at
  t=0 absorbs first-collective/CC-core startup latency behind compute.

Steady state is bound by the SWDGE gather descriptor rate (~34ns/descriptor
per DMA engine, ~204k gathered rows per layer per core).

Measured on 8xTRN2: 2.95 ms HW exec, rel err 8.5e-3 (prior session baseline
3.97 ms at rel err 4.6e-3; gate 2e-2).
"""
import os
import sys
import time
import hashlib

import numpy as np
import ml_dtypes

for _p in ("/opt/trn_rl_repo", "/root/.axon_site/_ro/trn_rl_repo"):
    if os.path.isdir(_p) and _p not in sys.path:
        sys.path.append(_p)

import concourse.bass as bass  # noqa: E402
import concourse.bacc as bacc  # noqa: E402
import concourse.mybir as mybir  # noqa: E402
from concourse import tile  # noqa: E402
from concourse import bass_utils  # noqa: E402

BF16 = mybir.dt.bfloat16
F32 = mybir.dt.float32
F8 = mybir.dt.float8e4
I16 = mybir.dt.int16
NPF8 = ml_dtypes.float8_e4m3

NCORES = 8
PT = 128
NQUEUES = 4
MAXC = 8  # chunks (x128 rows) per dma_gather call; 1024-idx HW cap
SINGLE_PACKET = bool(int(os.environ.get("GNN_SP", "1")))
QLVL = 12  # identity-chunk levels per (half, tile) cell
# strips of tiles; strip 0 is processed in two phases (table-0 then table-1)
# and is larger so its phase-A gathers absorb the end-of-layer AllGather
STRIPS = [(0, 24), (24, 40), (40, 49)]
# per-z-table fp8 scale: z values are multiplied by ALPHA[k] before the e4m3
# cast so each table's rms lands near 8 (e4m3 normal range, far from both
# the 448 saturation and the 2^-6 denormal floor)
ALPHA = [8.0, 256.0, 128.0, 1024.0, 256.0, 2048.0]


# ----------------------------------------------------------------------------
# host-side graph preprocessing (index-only)
# ----------------------------------------------------------------------------

def _wrap_idxs(idx_flat: np.ndarray) -> np.ndarray:
    """[n] int16 -> [128, n/16]: idx i at [i%16, i//16], replicated x8."""
    n = idx_flat.shape[0]
    assert n % 16 == 0
    w = idx_flat.reshape(n // 16, 16).T.astype(np.int16)
    return np.ascontiguousarray(np.tile(w, (8, 1)))


def _prep_graph(src: np.ndarray, dst: np.ndarray, n_nodes: int):
    """Bucket edges into identity-slot + overflow gather layout.

    z_own row layout (per core): row 0 = zero, rows 1..npadn = nodes
    0..npadn-1 (nshard real + pad nodes, forced to zero), row NROWS-1 junk.
    AG slab h covers rows [h*SLAB, (h+1)*SLAB); table-local index of node
    (c, n): c*SLAB + 1 + n (h=0, n < H0) or c*SLAB + n - H0 (h=1).
    """
    nshard = n_nodes // NCORES
    nt = (nshard + PT - 1) // PT
    npadn = nt * PT
    H0 = ((nshard // 2 + PT - 1) // PT) * PT   # 3136: half-0 node range
    SLAB = H0 + 1
    assert 2 * SLAB >= npadn + 1
    assert NCORES * SLAB - 1 <= np.iinfo(np.int16).max
    npadn_free = npadn - nshard  # pad-node rows usable as zeros (half 1)

    src = np.asarray(src, np.int64)
    dst = np.asarray(dst, np.int64)
    nonself = src != dst
    src, dst = src[nonself], dst[nonself]

    # ---- per core: identity slots + overflow rows ----
    # ident[c]: dict (h, t, lev, m) is implicit: array [2, nt, QLVL, PT] of
    # table-local idx (or -1). ovf[c]: dict (h, t) -> list of (tl, m1, m2).
    ident_idx = np.full((NCORES, 2, nt, QLVL, PT), -1, np.int64)
    ovf_rows = [dict() for _ in range(NCORES)]
    for c in range(NCORES):
        mask = (dst >= c * nshard) & (dst < (c + 1) * nshard)
        es = src[mask]
        ed = dst[mask] - c * nshard
        sc = es // nshard
        sn = es % nshard
        h = (sn >= H0).astype(np.int64)
        tl = sc * SLAB + np.where(h == 1, sn - H0, sn + 1)
        t = ed >> 7
        m = ed & 127
        cell = h * nt + t
        key = cell * n_nodes + es
        order = np.lexsort((m, key))
        key_s = key[order]
        m_s, tl_s, cell_s = m[order], tl[order], cell[order]
        bnd = np.flatnonzero(np.diff(key_s)) + 1
        starts = np.r_[0, bnd]
        ends = np.r_[bnd, len(key_s)]
        glen = ends - starts
        single = glen == 1
        # -- singles: rank within (cell, m), level < QLVL -> identity slot
        sp = starts[single]
        s_cell, s_m, s_tl = cell_s[sp], m_s[sp], tl_s[sp]
        so = np.lexsort((s_tl, s_m, s_cell))
        s_cell, s_m, s_tl = s_cell[so], s_m[so], s_tl[so]
        grp = s_cell * PT + s_m
        newg = np.r_[True, grp[1:] != grp[:-1]]
        gid = np.cumsum(newg) - 1
        gstart = np.flatnonzero(newg)
        lev = np.arange(len(grp)) - gstart[gid]
        put = lev < QLVL
        hh = s_cell[put] // nt
        tt = s_cell[put] % nt
        ident_idx[c, hh, tt, lev[put], s_m[put]] = s_tl[put]
        # -- overflow: deep singles (1-dst rows) + multi groups (paired)
        od = {}
        deep = ~put
        for cc, mm, tt2 in zip(s_cell[deep], s_m[deep], s_tl[deep]):
            od.setdefault((int(cc) // nt, int(cc) % nt), []) \
                .append((int(tt2), int(mm), -1))
        multi = np.flatnonzero(~single)
        for gi in multi:
            lo, hi = int(starts[gi]), int(ends[gi])
            cc = int(cell_s[lo])
            lst = od.setdefault((cc // nt, cc % nt), [])
            k = lo
            while k + 1 < hi:
                lst.append((int(tl_s[k]), int(m_s[k]), int(m_s[k + 1])))
                k += 2
            if k < hi:
                lst.append((int(tl_s[k]), int(m_s[k]), -1))
        for lst in od.values():
            lst.sort()  # src-sorted rows keep gather addresses ascending
        ovf_rows[c] = od

    # ---- uniform overflow layout (max rows across cores per cell) ----
    rows_ut = np.zeros((2, nt), np.int64)
    for h in range(2):
        for t in range(nt):
            rows_ut[h, t] = max(len(ovf_rows[c].get((h, t), []))
                                for c in range(NCORES))
    assert STRIPS[-1][1] == nt
    strip_end = {hi - 1 for (_, hi) in STRIPS}
    off = np.zeros((2, nt), np.int64)
    nch_ovf = [0, 0]
    rows_ext = rows_ut.copy()
    for h in range(2):
        acc = 0
        for t in range(nt):
            off[h, t] = acc
            acc += rows_ut[h, t]
            if t in strip_end:
                pad = (-acc) % PT
                rows_ext[h, t] = rows_ut[h, t] + pad
                acc += pad
        assert acc % PT == 0
        nch_ovf[h] = acc // PT

    # overflow jobs: per tile, ordered (h asc, chunk asc). job -> (h, c, t)
    ovf_jobs_by_tile = []
    njobs = 0
    for t in range(nt):
        jt = []
        for h in range(2):
            if rows_ut[h, t] == 0:
                continue
            lo, hi = off[h, t], off[h, t] + rows_ext[h, t]
            for c in range(int(lo) >> 7, int(hi - 1 >> 7) + 1):
                jt.append((h, c, njobs))
                njobs += 1
        ovf_jobs_by_tile.append(jt)

    # idx column layout: [h0 ident | h0 ovf | h1 ident | h1 ovf]
    nident = nt * QLVL
    base = [0, nident + nch_ovf[0]]
    ncols = 2 * nident + nch_ovf[0] + nch_ovf[1]

    mcols = np.arange(PT, dtype=np.int32)
    per_core = []
    for c in range(NCORES):
        idx_flat = np.empty(ncols * PT, np.int16)
        for h in range(2):
            lo = base[h] * PT
            nreg = nident + nch_ovf[h]
            # empty/pad slots gather a zero row, spread over cores (and the
            # 22 pad-node rows for half 1) to avoid a same-address hotspot
            ch = np.repeat(np.arange(nreg), PT)
            p = np.tile(np.arange(PT), nreg)
            core = (p + 3 * ch) % NCORES
            if h == 0:
                zfill = core * SLAB
            else:
                zfill = (core * SLAB + (nshard - H0)
                         + (p * 7 + ch) % npadn_free)
            idx_flat[lo:lo + nreg * PT] = zfill.astype(np.int16)
            ii = ident_idx[c, h].reshape(nident, PT)
            reg = idx_flat[lo:lo + nident * PT].reshape(nident, PT)
            np.copyto(reg, ii.astype(np.int16), where=ii >= 0)
        d1 = np.full((PT, njobs), -1, np.int32)
        d2 = np.full((PT, njobs), -1, np.int32)
        for t in range(nt):
            for (h, ch, j) in ovf_jobs_by_tile[t]:
                lst = ovf_rows[c].get((h, t), [])
                lo = int(off[h, t])
                obase = (base[h] + nident) * PT
                for p in range(PT):
                    r = ch * PT + p - lo  # row index within the cell
                    if 0 <= r < len(lst):
                        tl, m1, m2 = lst[r]
                        idx_flat[obase + ch * PT + p] = tl
                        d1[p, j] = m1
                        if m2 >= 0:
                            d2[p, j] = m2
        smat = ((d1[:, :, None] == mcols).astype(np.int8)
                + (d2[:, :, None] == mcols).astype(np.int8)).astype(NPF8)
        per_core.append(dict(
            idx=_wrap_idxs(idx_flat),
            smat=np.ascontiguousarray(smat),
        ))

    # per-(strip, half) overflow chunk ranges
    strip_ovf = []
    for (s0, s1) in STRIPS:
        t_last = s1 - 1
        strip_ovf.append(tuple(
            (int(off[h, s0]) // PT,
             int(off[h, t_last] + rows_ext[h, t_last]) // PT)
            for h in range(2)))

    meta = dict(nshard=nshard, nt=nt, H0=H0, SLAB=SLAB, npadn=npadn,
                nch_ovf=nch_ovf, nident=nident, base=base, ncols=ncols,
                ovf_jobs_by_tile=ovf_jobs_by_tile, njobs=njobs,
                strip_ovf=strip_ovf)
    return meta, per_core


def _zrowb(dz: int) -> int:
    """fp8 z-table row bytes, padded to the 256B dma_gather granule."""
    return ((dz + 255) // 256) * 256


# ----------------------------------------------------------------------------
# device program builder
# ----------------------------------------------------------------------------

def _build_program(meta, dims, fc_out):
    nshard, nt = meta["nshard"], meta["nt"]
    H0, SLAB, npadn = meta["H0"], meta["SLAB"], meta["npadn"]
    nch_ovf, nident, base = meta["nch_ovf"], meta["nident"], meta["base"]
    ncols, njobs = meta["ncols"], meta["njobs"]
    ovf_jobs_by_tile = meta["ovf_jobs_by_tile"]
    din0 = dims[0][0]
    NROWS = 2 * SLAB

    wdims = []
    for (di, do) in dims:
        wdims.append((di, do))   # sage
        wdims.append((do, do))   # gc
    wdims.append((dims[-1][1], fc_out))  # fc
    # layer 0 aggregates raw features (din0); layers 1..5 aggregate z_k
    agg_dz = [din0] + [wdims[k][1] for k in range(1, 6)]
    zrowb = [_zrowb(d) for d in agg_dz]

    rows = [PT] * nt
    rows[nt - 1] = nshard - (nt - 1) * PT

    nc = bacc.Bacc("TRN2", target_bir_lowering=False, debug=False,
                   num_devices=NCORES, num_swdge_queues=NQUEUES)

    # fp8 feature tables: layer 0 gathers features directly (W0 applied
    # after aggregation), so no z0 table and no layer-0 AllGather. fh0/fh1
    # are host-replicated images of what the AG would have produced;
    # feat_own mirrors the z_own row layout for the local term.
    fh_in = [nc.dram_tensor(f"fh{h}", [NCORES * SLAB, din0], F8,
                            kind="ExternalInput") for h in (0, 1)]
    fown_in = nc.dram_tensor("fown", [NROWS, din0], F8, kind="ExternalInput")
    idx_in = nc.dram_tensor("idx", [PT, ncols * 8], I16,
                            kind="ExternalInput")
    smat_in = nc.dram_tensor("smat", [PT, njobs, PT], F8,
                             kind="ExternalInput")
    ii_in = nc.dram_tensor("ii", [PT, 2, PT], F8, kind="ExternalInput")
    twoi_in = nc.dram_tensor("twoi", [PT, PT], F8, kind="ExternalInput")
    ident_in = nc.dram_tensor("ident", [PT, PT], BF16, kind="ExternalInput")
    zscale_in = nc.dram_tensor("zscale", [PT, 6, nt], F32,
                               kind="ExternalInput")
    escale_in = nc.dram_tensor("escale", [PT, 6, nt], F32,
                               kind="ExternalInput")
    w_in = []
    for k, (di, do) in enumerate(wdims):
        w_in.append(nc.dram_tensor(f"w{k}", [PT, di // PT, do], BF16,
                                   kind="ExternalInput"))
    out = nc.dram_tensor("out", [nshard, fc_out], F32, kind="ExternalOutput")

    with tile.TileContext(nc) as tcx:
        with (
            tcx.tile_pool(name="const", bufs=1) as constp,
            tcx.tile_pool(name="g", bufs=14) as gp,
            tcx.tile_pool(name="zt", bufs=3) as ztp,
            tcx.tile_pool(name="pa", bufs=2) as pap,
            tcx.tile_pool(name="pf", bufs=2) as pfp,
            tcx.tile_pool(name="u", bufs=2) as up,
            tcx.tile_pool(name="h", bufs=3) as hp,
            tcx.tile_pool(name="ht", bufs=2) as htp,
            tcx.tile_pool(name="zo", bufs=3) as zop,
            tcx.tile_pool(name="z0", bufs=1) as z0p,
            tcx.tile_pool(name="aggps", bufs=3, space="PSUM") as aggpsp,
            tcx.tile_pool(name="trps", bufs=2, space="PSUM") as trpsp,
            tcx.tile_pool(name="mmps", bufs=2, space="PSUM") as mmpsp,
            tcx.tile_pool(name="dram", bufs=1, space="DRAM") as dramp,
        ):
            # tiny dummy AllGather issued first: absorbs first-collective
            # warmup / core-launch skew while stage B0 runs
            warm_in = dramp.tile([16, 256], F8, name="warm_in")
            warm_out = dramp.tile([NCORES * 16, 256], F8, name="warm_out",
                                  addr_space="Shared")
            nc.gpsimd.collective_compute(
                "AllGather", mybir.AluOpType.bypass,
                replica_groups=[list(range(NCORES))],
                ins=[warm_in[:].opt()], outs=[warm_out[:].opt()])

            # persistent constants (smat pinned for all 6 layers)
            idx_sb = constp.tile([PT, ncols * 8], I16, name="idx_sb")
            nc.sync.dma_start(idx_sb[:], idx_in[:])
            smat_sb = constp.tile([PT, njobs, PT], F8, name="smat_sb")
            nc.sync.dma_start(smat_sb[:], smat_in[:])
            ii_sb = constp.tile([PT, 2, PT], F8, name="ii_sb")
            nc.sync.dma_start(ii_sb[:], ii_in[:])
            twoi_sb = constp.tile([PT, PT], F8, name="twoi_sb")
            nc.sync.dma_start(twoi_sb[:], twoi_in[:])
            ident_sb = constp.tile([PT, PT], BF16, name="ident_sb")
            nc.sync.dma_start(ident_sb[:], ident_in[:])
            zscale_sb = constp.tile([PT, 6, nt], F32, name="zscale_sb")
            nc.sync.dma_start(zscale_sb[:], zscale_in[:])
            escale_sb = constp.tile([PT, 6, nt], F32, name="escale_sb")
            nc.sync.dma_start(escale_sb[:], escale_in[:])
            w_sb = []
            for k, (di, do) in enumerate(wdims):
                w = constp.tile([PT, di // PT, do], BF16, name=f"w{k}_sb")
                nc.sync.dma_start(w[:], w_in[k][:])
                w_sb.append(w)

            # internal DRAM: z shards + gathered half tables (k=1..5 only;
            # layer 0 reads the fh/fown inputs). Row 0 of each z_own is the
            # half-0 zero row; written once here.
            z_own, z_half = [None], [None]
            zseed = z0p.tile([PT, max(zrowb)], F8, name="zseed")
            nc.vector.memset(zseed[:], 0.0)
            for a in range(1, 6):
                zb = zrowb[a]
                z_own.append(dramp.tile([NROWS, zb], F8, name=f"zown{a}"))
                z_half.append([
                    dramp.tile([NCORES * SLAB, zb], F8,
                               name=f"zhalf{a}_{h}", addr_space="Shared")
                    for h in (0, 1)])
                nc.sync.dma_start(z_own[a][0:1, :], zseed[0:1, :zb])

            def run_ag(a, h):
                nc.gpsimd.collective_compute(
                    "AllGather", mybir.AluOpType.bypass,
                    replica_groups=[list(range(NCORES))],
                    ins=[z_own[a][h * SLAB:(h + 1) * SLAB, :].opt()],
                    outs=[z_half[a][h][:].opt()],
                )

            def stage_b(t, h_tile, k, dest, dest_dtype):
                """h_tile [128, Din_k] bf16 -> dest rows of tile t."""
                di, do = wdims[k]
                kg = di // PT
                ht = htp.tile([PT, kg, PT], BF16, tag="ht")
                for g in range(kg):
                    trp = trpsp.tile([PT, PT], BF16, tag="tr")
                    nc.tensor.transpose(trp[:], h_tile[:, g * PT:(g + 1) * PT],
                                        ident_sb[:])
                    nc.vector.tensor_copy(ht[:, g, :], trp[:])
                zp = mmpsp.tile([PT, do], F32, tag="mm")
                for g in range(kg):
                    nc.tensor.matmul(zp[:], ht[:, g, :], w_sb[k][:, g, :],
                                     start=(g == 0), stop=(g == kg - 1))
                zo = zop.tile([PT, do], dest_dtype, tag="zo")
                if dest_dtype == F32:  # fc output: unpadded dest
                    nc.vector.tensor_copy(zo[:], zp[:])
                    nc.sync.dma_start(dest[t * PT:t * PT + rows[t], :],
                                      zo[:rows[t], :])
                else:  # z table: fp8 cast with per-node scale; +1 row offset
                    nc.scalar.activation(zo[:], zp[:],
                                         mybir.ActivationFunctionType.Copy,
                                         scale=zscale_sb[:, k, t:t + 1])
                    nc.sync.dma_start(
                        dest[1 + t * PT:1 + (t + 1) * PT, :do], zo[:])

            ag_t = (H0 + PT - 1) // PT - 1  # slab 0 complete after this tile

            # ---- 6 aggregation layers + following stage B ----
            call_id = 0
            for a in range(6):
                dz = agg_dz[a]
                zb = zrowb[a]
                sage = (a % 2 == 0)
                k_next = a + 1
                is_fc = (k_next == 6)

                chunkmap = {}   # (kind, h, chunk) -> (g tile, slot)
                cursor = {}     # (kind, si, h) -> next chunk to emit
                ztab = fh_in if a == 0 else z_half[a]
                zown_src = fown_in if a == 0 else z_own[a]

                def emit_calls(kind, si, h, upto_chunk):
                    """kind 'i': ident chunks (strip range [s0*Q, s1*Q));
                    kind 'o': overflow chunks (meta strip ranges)."""
                    nonlocal call_id
                    if kind == "i":
                        s0, s1 = STRIPS[si]
                        c_lo, c_hi = s0 * QLVL, s1 * QLVL
                        col0 = base[h]
                    else:
                        c_lo, c_hi = meta["strip_ovf"][si][h]
                        col0 = base[h] + nident
                    cur = cursor.setdefault((kind, si, h), c_lo)
                    while cur <= upto_chunk and cur < c_hi:
                        cw = min(MAXC, c_hi - cur)
                        g = gp.tile([PT, MAXC, zb], F8, tag="g")
                        col = col0 + cur
                        nc.gpsimd.dma_gather(
                            g[:, :cw, :], ztab[h][:],
                            idx_sb[:, col * 8:(col + cw) * 8],
                            cw * PT, cw * PT, zb,
                            single_packet=SINGLE_PACKET,
                            queue_num=call_id % NQUEUES)
                        for k in range(cw):
                            chunkmap[(kind, h, cur + k)] = (g, k)
                        cur += cw
                        cursor[(kind, si, h)] = cur
                        call_id += 1

                def agg_matmuls(aggp, jt, first_starts):
                    """jt: list of (kind, h, c, sji); sji = smat job index
                    for 'o' jobs (ignored for 'i'). Consecutive same-call
                    chunk pairs run as one DoubleRow matmul."""
                    nj = len(jt)
                    i = 0
                    first = True
                    while i < nj:
                        kind, h, c, sji = jt[i]
                        gt, slot = chunkmap[(kind, h, c)]
                        w = 1
                        if i + 1 < nj:
                            k2, h2, c2, sji2 = jt[i + 1]
                            if (k2 == kind and h2 == h and c2 == c + 1
                                    and (kind == "i" or sji2 == sji + 1)):
                                gt2, slot2 = chunkmap[(kind, h2, c2)]
                                if gt2 is gt and slot2 == slot + 1:
                                    w = 2
                        start = first and first_starts
                        stop = (i + w == nj)
                        if w == 2:
                            lhs2 = (ii_sb[:, 0:2, :] if kind == "i"
                                    else smat_sb[:, sji:sji + 2, :])
                            nc.tensor.matmul(
                                aggp[:], lhs2, gt[:, slot:slot + 2, :dz],
                                start=start, stop=stop,
                                perf_mode=mybir.MatmulPerfMode.DoubleRow)
                        else:
                            lhs1 = (ii_sb[:, 0, :] if kind == "i"
                                    else smat_sb[:, sji, :])
                            nc.tensor.matmul(
                                aggp[:], lhs1, gt[:, slot, :dz],
                                start=start, stop=stop)
                        first = False
                        i += w

                def tile_jobs(t, want_h):
                    """Job list for tile t restricted to half want_h (or both
                    if None): ident jobs then overflow jobs."""
                    jt = []
                    for h in (0, 1):
                        if want_h is not None and h != want_h:
                            continue
                        for c in range(t * QLVL, (t + 1) * QLVL):
                            jt.append(("i", h, c, -1))
                    for (h, c, j) in ovf_jobs_by_tile[t]:
                        if want_h is not None and h != want_h:
                            continue
                        jt.append(("o", h, c, j))
                    return jt

                def emit_for(t, si, want_h):
                    for h in (0, 1):
                        if want_h is not None and h != want_h:
                            continue
                        emit_calls("i", si, h, (t + 1) * QLVL - 1)
                        oc = [c for (hh, c, _) in ovf_jobs_by_tile[t]
                              if hh == h]
                        if oc:
                            emit_calls("o", si, h, max(oc))

                def local_mms(t, aggp, zt):
                    lhs = twoi_sb[:] if sage else ii_sb[:, 0, :]
                    nc.tensor.matmul(aggp[:], lhs, zt[:, :dz],
                                     start=True, stop=False)

                def epilogue(t, src_ap):
                    """relu(scale * src) -> stage_b -> (maybe) first AG."""
                    if a == 0:
                        # X = agg/(den*alpha_f); h1 = relu(X @ W0^T) -> z1
                        di, do = wdims[0]
                        X = hp.tile([PT, di], BF16, tag="h")
                        nc.scalar.activation(X[:], src_ap,
                                             mybir.ActivationFunctionType.Copy,
                                             scale=escale_sb[:, 0, t:t + 1])
                        kg = di // PT
                        ht = htp.tile([PT, kg, PT], BF16, tag="ht")
                        for g in range(kg):
                            trp = trpsp.tile([PT, PT], BF16, tag="tr")
                            nc.tensor.transpose(trp[:],
                                                X[:, g * PT:(g + 1) * PT],
                                                ident_sb[:])
                            nc.vector.tensor_copy(ht[:, g, :], trp[:])
                        zp = mmpsp.tile([PT, do], F32, tag="mm")
                        for g in range(kg):
                            nc.tensor.matmul(zp[:], ht[:, g, :],
                                             w_sb[0][:, g, :],
                                             start=(g == 0),
                                             stop=(g == kg - 1))
                        hx = hp.tile([PT, do], BF16, tag="h")
                        nc.scalar.activation(hx[:], zp[:],
                                             mybir.ActivationFunctionType.Relu)
                        stage_b(t, hx, 1, z_own[1], F8)
                        if t == ag_t:
                            run_ag(1, 0)
                        return
                    hx = hp.tile([PT, dz], BF16, tag="h")
                    nc.scalar.activation(hx[:], src_ap,
                                         mybir.ActivationFunctionType.Relu,
                                         scale=escale_sb[:, a, t:t + 1])
                    if is_fc:
                        stage_b(t, hx, 6, out, F32)
                    else:
                        stage_b(t, hx, k_next, z_own[k_next], F8)
                        if t == ag_t:
                            run_ag(k_next, 0)

                def single_pass(t, si):
                    emit_for(t, si, None)
                    zt = ztp.tile([PT, zb], F8, tag="zt")
                    nc.sync.dma_start(
                        zt[:], zown_src[1 + t * PT:1 + (t + 1) * PT, :])
                    aggp = aggpsp.tile([PT, dz], F32, tag="agg")
                    local_mms(t, aggp, zt)
                    agg_matmuls(aggp, tile_jobs(t, None), False)
                    epilogue(t, aggp[:])

                # Layer 0's tables are inputs (no AllGather to hide):
                # single-pass everywhere. Later layers run strip 0 in two
                # phases (table-0 only, then table-1) to absorb the
                # end-of-previous-layer AllGather of table 1.
                if a == 0:
                    for si in range(len(STRIPS)):
                        for t in range(*STRIPS[si]):
                            single_pass(t, si)
                else:
                    tiles0 = list(range(*STRIPS[0]))
                    pa = pap.tile([PT, len(tiles0), dz], BF16, tag="pa")
                    for i, t in enumerate(tiles0):
                        emit_for(t, 0, 0)
                        zt = ztp.tile([PT, zb], F8, tag="zt")
                        nc.sync.dma_start(
                            zt[:], zown_src[1 + t * PT:1 + (t + 1) * PT, :])
                        aggp = aggpsp.tile([PT, dz], F32, tag="agg")
                        local_mms(t, aggp, zt)
                        agg_matmuls(aggp, tile_jobs(t, 0), False)
                        nc.scalar.activation(pa[:, i, :], aggp[:],
                                             mybir.ActivationFunctionType.Copy)
                    for i, t in enumerate(tiles0):
                        emit_for(t, 0, 1)
                        aggp = aggpsp.tile([PT, dz], F32, tag="agg")
                        agg_matmuls(aggp, tile_jobs(t, 1), True)
                        pf = pfp.tile([PT, dz], F32, tag="pf")
                        nc.scalar.activation(pf[:], pa[:, i, :],
                                             mybir.ActivationFunctionType.Copy)
                        u = up.tile([PT, dz], F32, tag="u")
                        nc.vector.tensor_tensor(u[:], aggp[:], pf[:],
                                                mybir.AluOpType.add)
                        epilogue(t, u[:])
                    for si in range(1, len(STRIPS)):
                        for t in range(*STRIPS[si]):
                            single_pass(t, si)
                if not is_fc:
                    run_ag(k_next, 1)

    t0 = time.time()
    nc.compile()
    print(f"[kernel] bacc compile: {time.time() - t0:.1f}s", file=sys.stderr)
    return nc


# ----------------------------------------------------------------------------
# public entry
# ----------------------------------------------------------------------------

_CACHE = {}


def _build_in_maps(features, src, dst, sage_ws, gc_ws, fc_w, meta, per_core):
    n_nodes, din0 = features.shape
    nshard, nt = meta["nshard"], meta["nt"]

    e_ones = np.ones(len(src), np.float64)
    in_deg = np.bincount(dst, weights=e_ones, minlength=n_nodes)
    out_deg = np.bincount(src, weights=e_ones, minlength=n_nodes)
    inv_den = (1.0 / (in_deg + 1.0)).astype(np.float32)
    inv_dst = (np.where(in_deg > 0, in_deg, 1.0) ** -0.5).astype(np.float32)
    inv_src = (np.where(out_deg > 0, out_deg, 1.0) ** -0.5).astype(np.float32)

    def shard_scal(v, c):
        s = np.ones(nt * PT, np.float32)
        s[:nshard] = v[c * nshard:(c + 1) * nshard]
        return np.ascontiguousarray(s.reshape(nt, PT).T)

    worder = []
    for s in range(len(sage_ws)):
        worder.append(sage_ws[s])
        worder.append(gc_ws[s])
    worder.append(fc_w)
    w_arrs = []
    for w in worder:
        do, di = w.shape
        wt = np.ascontiguousarray(
            w.T.astype(np.float32).reshape(di // PT, PT, do)
            .transpose(1, 0, 2)).astype(ml_dtypes.bfloat16)
        w_arrs.append(wt)

    ident = np.eye(PT, dtype=np.float32).astype(ml_dtypes.bfloat16)
    eye = np.eye(PT, dtype=np.float32)
    ii = np.ascontiguousarray(
        np.stack([eye, eye], axis=1).astype(NPF8))       # [PT, 2, PT]
    twoi = np.ascontiguousarray((2.0 * eye).astype(NPF8))

    # fp8 feature tables (alpha-scaled): fh0/fh1 replicate every core's
    # shard-half (z_half layout, incl zero rows); fown mirrors z_own rows.
    H0, SLAB = meta["H0"], meta["SLAB"]
    NROWS = 2 * SLAB
    f8 = (features.astype(np.float32) * ALPHA[0]).astype(NPF8)
    fh = [np.zeros((NCORES * SLAB, din0), NPF8) for _ in range(2)]
    fown_all = []
    for c in range(NCORES):
        sh = f8[c * nshard:(c + 1) * nshard]
        fh[0][c * SLAB + 1:c * SLAB + 1 + H0] = sh[:H0]
        fh[1][c * SLAB:c * SLAB + nshard - H0] = sh[H0:]
        fo = np.zeros((NROWS, din0), NPF8)
        fo[1:1 + nshard] = sh
        fown_all.append(fo)
    fh = [np.ascontiguousarray(x) for x in fh]

    # pad-node rows (t = nt-1, partitions >= rows[-1]) must produce z = 0:
    # zero their epilogue scale so hx and every later z stay 0 (gathered
    # zero rows and the AG slab-1 zero row depend on it)
    rows_last = nshard - (nt - 1) * PT
    in_maps = []
    for c in range(NCORES):
        zscale = np.empty((PT, 6, nt), np.float32)
        escale = np.empty((PT, 6, nt), np.float32)
        isrc = shard_scal(inv_src, c)
        iden = shard_scal(inv_den, c)
        idst = shard_scal(inv_dst, c)
        for k in range(6):
            zscale[:, k, :] = ALPHA[k] * (isrc if k % 2 == 1 else 1.0)
            escale[:, k, :] = (iden if k % 2 == 0 else idst) / ALPHA[k]
        escale[rows_last:, :, nt - 1] = 0.0
        im = dict(
            fh0=fh[0],
            fh1=fh[1],
            fown=fown_all[c],
            idx=per_core[c]["idx"],
            smat=per_core[c]["smat"],
            ii=ii,
            twoi=twoi,
            ident=ident,
            zscale=zscale,
            escale=escale,
        )
        for k, w in enumerate(w_arrs):
            im[f"w{k}"] = w
        in_maps.append(im)
    return in_maps


def _run(features, src, dst, sage_ws, sage_bs, gc_ws, gc_bs, fc_w, fc_b):
    n_nodes, din0 = features.shape
    dims = [(w.shape[1], w.shape[0]) for w in sage_ws]
    fc_out = fc_w.shape[0]

    key = hashlib.sha1(
        np.asarray(src).tobytes() + np.asarray(dst).tobytes()
        + str((n_nodes, din0, dims, fc_out)).encode()
    ).hexdigest()
    if key in _CACHE:
        nc, meta, per_core = _CACHE[key]
    else:
        meta, per_core = _prep_graph(src, dst, n_nodes)
        nc = _build_program(meta, dims, fc_out)
        _CACHE[key] = (nc, meta, per_core)

    biases = list(sage_bs) + list(gc_bs) + [fc_b]
    if any(np.any(np.asarray(b) != 0) for b in biases):
        raise NotImplementedError("nonzero biases not supported")

    in_maps = _build_in_maps(features, src, dst, sage_ws, gc_ws, fc_w,
                             meta, per_core)

    trace = bool(int(os.environ.get("GNN_TRACE", "0")))
    res = bass_utils.run_bass_kernel_spmd(
        nc, in_maps, core_ids=list(range(NCORES)), trace=trace)
    out = np.concatenate([res.results[c]["out"] for c in range(NCORES)],
                         axis=0).astype(np.float32)
    if trace:
        print(f"[kernel] exec_time_ns: {res.exec_time_ns}", file=sys.stderr)
        _CACHE["last_exec_time_ns"] = res.exec_time_ns
        _CACHE["last_profile"] = res.profile_json
    return out


def kernel(features, src, dst,
           sage_w0, sage_b0, gc_w0, gc_b0,
           sage_w1, sage_b1, gc_w1, gc_b1,
           sage_w2, sage_b2, gc_w2, gc_b2,
           fc_w, fc_b):
    features = np.asarray(features, np.float32)
    src = np.asarray(src, np.int64)
    dst = np.asarray(dst, np.int64)
    return _run(
        features, src, dst,
        [np.asarray(sage_w0), np.asarray(sage_w1), np.asarray(sage_w2)],
        [np.asarray(sage_b0), np.asarray(sage_b1), np.asarray(sage_b2)],
        [np.asarray(gc_w0), np.asarray(gc_w1), np.asarray(gc_w2)],
        [np.asarray(gc_b0), np.asarray(gc_b1), np.asarray(gc_b2)],
        np.asarray(fc_w), np.asarray(fc_b),
    )


# revision 31
# speedup vs baseline: 1.1728x; 1.1728x over previous
"""GNN message-passing (SAGE-gcn + GraphConv stack) Trainium2 Bass kernel.

Strategy (8 NeuronCores, dst-node sharded):
- Each core owns a contiguous block of N/8 dst nodes and all edges into them.
- Per layer, the small weight matmul is applied BEFORE aggregation
  (A @ (h W^T) == (A @ h) W^T), shrinking the gathered message dim.
- z tables are stored in float8e4 with per-layer power-of-2 scales (folded
  into host-provided cast/epilogue scale tables); rows padded to 256B
  multiples (dma_gather constraint). Halves gather DMA bytes on the wide
  layers and enables DoubleRow fp8 matmuls (2 chunks per instruction).
- Each core computes z for its own nodes; an in-kernel AllGather replicates
  the full z table (fp8) into every core's DRAM. Each AG slab carries a
  dedicated all-zero row (row 0 of half 0; the first pad-node row of half 1)
  used as the gather target for empty identity slots.
- Aggregation, identity-chunk layout: per (half, dst-tile) cell, edges whose
  src has a single edge into the cell are placed at gather position = their
  dst slot, Q=12 levels deep -> the one-hot S for those chunks is a CONSTANT
  identity (no S traffic at all). Multi-edge srcs (dedup pairs riding a
  merged 0/1/2 column) and singles deeper than Q go to a small "overflow"
  region with host-precomputed fp8 S matrices, small enough to PIN in SBUF
  across all 6 layers. Consecutive chunk pairs run as one DoubleRow matmul.
- Self-loops are removed from the gather stream. The sage "+h" term and the
  self-loop contribution ride a per-tile "local chunk" (contiguous HWDGE load
  of the core's own z rows) multiplied by 2I (sage) or I (gc).
- Host does index-only preprocessing: edge bucketing, identity-slot layout,
  int16 gather indices (two ~25k-row half tables), degree scalars, S tables.
- The two gather tables are node-range halves (n < 3136 within the shard) so
  table 0's AllGather fires mid-layer (after tile 24's stage_b) and overlaps
  compute; only table 1's AllGather sits at the layer end, and strip 0 of the
  next layer runs phase-A (table-0 jobs only, partial spilled to SBUF bf16)
  to absorb its latency.
- Layer 0 gathers raw fp8 features from host-replicated half tables (same
  layout the AG would produce) and applies W0 after aggregation, removing
  stage B0's AllGathers from the critical path; a tiny warmup AllGather at
  t=0 absorbs first-collective/CC-core startup latency behind compute.

Steady state is bound by the SWDGE gather descriptor rate (~34ns/descriptor
per DMA engine, ~204k gathered rows per layer per core).

Measured on 8xTRN2: 2.95 ms HW exec, rel err 8.5e-3 (prior session baseline
3.97 ms at rel err 4.6e-3; gate 2e-2).
"""
import os
import sys
import time
import hashlib

import numpy as np
import ml_dtypes

for _p in ("/opt/trn_rl_repo", "/root/.axon_site/_ro/trn_rl_repo"):
    if os.path.isdir(_p) and _p not in sys.path:
        sys.path.append(_p)

import concourse.bass as bass  # noqa: E402
import concourse.bacc as bacc  # noqa: E402
import concourse.mybir as mybir  # noqa: E402
from concourse import tile  # noqa: E402
from concourse import bass_utils  # noqa: E402

BF16 = mybir.dt.bfloat16
F32 = mybir.dt.float32
F8 = mybir.dt.float8e4
I16 = mybir.dt.int16
NPF8 = ml_dtypes.float8_e4m3

NCORES = 8
PT = 128
NQUEUES = 4
MAXC = 8  # chunks (x128 rows) per dma_gather call; 1024-idx HW cap
SINGLE_PACKET = bool(int(os.environ.get("GNN_SP", "1")))
QLVL = 12  # identity-chunk levels per (half, tile) cell
# strips of tiles; strip 0 is processed in two phases (table-0 then table-1)
# and is larger so its phase-A gathers absorb the end-of-layer AllGather
STRIPS = [(0, 24), (24, 40), (40, 49)]
# per-z-table fp8 scale: z values are multiplied by ALPHA[k] before the e4m3
# cast so each table's rms lands near 8 (e4m3 normal range, far from both
# the 448 saturation and the 2^-6 denormal floor)
ALPHA = [8.0, 256.0, 128.0, 1024.0, 256.0, 2048.0]


# ----------------------------------------------------------------------------
# host-side graph preprocessing (index-only)
# ----------------------------------------------------------------------------

def _wrap_idxs(idx_flat: np.ndarray) -> np.ndarray:
    """[n] int16 -> [128, n/16]: idx i at [i%16, i//16], replicated x8."""
    n = idx_flat.shape[0]
    assert n % 16 == 0
    w = idx_flat.reshape(n // 16, 16).T.astype(np.int16)
    return np.ascontiguousarray(np.tile(w, (8, 1)))


def _prep_graph(src: np.ndarray, dst: np.ndarray, n_nodes: int):
    """Bucket edges into identity-slot + overflow gather layout.

    z_own row layout (per core): row 0 = zero, rows 1..npadn = nodes
    0..npadn-1 (nshard real + pad nodes, forced to zero), row NROWS-1 junk.
    AG slab h covers rows [h*SLAB, (h+1)*SLAB); table-local index of node
    (c, n): c*SLAB + 1 + n (h=0, n < H0) or c*SLAB + n - H0 (h=1).
    """
    nshard = n_nodes // NCORES
    nt = (nshard + PT - 1) // PT
    npadn = nt * PT
    H0 = ((nshard // 2 + PT - 1) // PT) * PT   # 3136: half-0 node range
    SLAB = H0 + 1
    assert 2 * SLAB >= npadn + 1
    assert NCORES * SLAB - 1 <= np.iinfo(np.int16).max
    npadn_free = npadn - nshard  # pad-node rows usable as zeros (half 1)

    src = np.asarray(src, np.int64)
    dst = np.asarray(dst, np.int64)
    nonself = src != dst
    src, dst = src[nonself], dst[nonself]

    # ---- per core: identity slots + overflow rows ----
    # ident[c]: dict (h, t, lev, m) is implicit: array [2, nt, QLVL, PT] of
    # table-local idx (or -1). ovf[c]: dict (h, t) -> list of (tl, m1, m2).
    ident_idx = np.full((NCORES, 2, nt, QLVL, PT), -1, np.int64)
    ovf_rows = [dict() for _ in range(NCORES)]
    for c in range(NCORES):
        mask = (dst >= c * nshard) & (dst < (c + 1) * nshard)
        es = src[mask]
        ed = dst[mask] - c * nshard
        sc = es // nshard
        sn = es % nshard
        h = (sn >= H0).astype(np.int64)
        tl = sc * SLAB + np.where(h == 1, sn - H0, sn + 1)
        t = ed >> 7
        m = ed & 127
        cell = h * nt + t
        key = cell * n_nodes + es
        order = np.lexsort((m, key))
        key_s = key[order]
        m_s, tl_s, cell_s = m[order], tl[order], cell[order]
        bnd = np.flatnonzero(np.diff(key_s)) + 1
        starts = np.r_[0, bnd]
        ends = np.r_[bnd, len(key_s)]
        glen = ends - starts
        single = glen == 1
        # -- singles: rank within (cell, m), level < QLVL -> identity slot
        sp = starts[single]
        s_cell, s_m, s_tl = cell_s[sp], m_s[sp], tl_s[sp]
        so = np.lexsort((s_tl, s_m, s_cell))
        s_cell, s_m, s_tl = s_cell[so], s_m[so], s_tl[so]
        grp = s_cell * PT + s_m
        newg = np.r_[True, grp[1:] != grp[:-1]]
        gid = np.cumsum(newg) - 1
        gstart = np.flatnonzero(newg)
        lev = np.arange(len(grp)) - gstart[gid]
        put = lev < QLVL
        hh = s_cell[put] // nt
        tt = s_cell[put] % nt
        ident_idx[c, hh, tt, lev[put], s_m[put]] = s_tl[put]
        # -- overflow: deep singles (1-dst rows) + multi groups (paired)
        od = {}
        deep = ~put
        for cc, mm, tt2 in zip(s_cell[deep], s_m[deep], s_tl[deep]):
            od.setdefault((int(cc) // nt, int(cc) % nt), []) \
                .append((int(tt2), int(mm), -1))
        multi = np.flatnonzero(~single)
        for gi in multi:
            lo, hi = int(starts[gi]), int(ends[gi])
            cc = int(cell_s[lo])
            lst = od.setdefault((cc // nt, cc % nt), [])
            k = lo
            while k + 1 < hi:
                lst.append((int(tl_s[k]), int(m_s[k]), int(m_s[k + 1])))
                k += 2
            if k < hi:
                lst.append((int(tl_s[k]), int(m_s[k]), -1))
        for lst in od.values():
            lst.sort()  # src-sorted rows keep gather addresses ascending
        ovf_rows[c] = od

    # ---- uniform overflow layout (max rows across cores per cell) ----
    rows_ut = np.zeros((2, nt), np.int64)
    for h in range(2):
        for t in range(nt):
            rows_ut[h, t] = max(len(ovf_rows[c].get((h, t), []))
                                for c in range(NCORES))
    assert STRIPS[-1][1] == nt
    strip_end = {hi - 1 for (_, hi) in STRIPS}
    off = np.zeros((2, nt), np.int64)
    nch_ovf = [0, 0]
    rows_ext = rows_ut.copy()
    for h in range(2):
        acc = 0
        for t in range(nt):
            off[h, t] = acc
            acc += rows_ut[h, t]
            if t in strip_end:
                pad = (-acc) % PT
                rows_ext[h, t] = rows_ut[h, t] + pad
                acc += pad
        assert acc % PT == 0
        nch_ovf[h] = acc // PT

    # overflow jobs: per tile, ordered (h asc, chunk asc). job -> (h, c, t)
    ovf_jobs_by_tile = []
    njobs = 0
    for t in range(nt):
        jt = []
        for h in range(2):
            if rows_ut[h, t] == 0:
                continue
            lo, hi = off[h, t], off[h, t] + rows_ext[h, t]
            for c in range(int(lo) >> 7, int(hi - 1 >> 7) + 1):
                jt.append((h, c, njobs))
                njobs += 1
        ovf_jobs_by_tile.append(jt)

    # idx column layout: [h0 ident | h0 ovf | h1 ident | h1 ovf]
    nident = nt * QLVL
    base = [0, nident + nch_ovf[0]]
    ncols = 2 * nident + nch_ovf[0] + nch_ovf[1]

    mcols = np.arange(PT, dtype=np.int32)
    per_core = []
    for c in range(NCORES):
        idx_flat = np.empty(ncols * PT, np.int16)
        for h in range(2):
            lo = base[h] * PT
            nreg = nident + nch_ovf[h]
            # empty/pad slots gather a zero row, spread over cores (and the
            # 22 pad-node rows for half 1) to avoid a same-address hotspot
            ch = np.repeat(np.arange(nreg), PT)
            p = np.tile(np.arange(PT), nreg)
            core = (p + 3 * ch) % NCORES
            if h == 0:
                zfill = core * SLAB
            else:
                zfill = (core * SLAB + (nshard - H0)
                         + (p * 7 + ch) % npadn_free)
            idx_flat[lo:lo + nreg * PT] = zfill.astype(np.int16)
            ii = ident_idx[c, h].reshape(nident, PT)
            reg = idx_flat[lo:lo + nident * PT].reshape(nident, PT)
            np.copyto(reg, ii.astype(np.int16), where=ii >= 0)
        d1 = np.full((PT, njobs), -1, np.int32)
        d2 = np.full((PT, njobs), -1, np.int32)
        for t in range(nt):
            for (h, ch, j) in ovf_jobs_by_tile[t]:
                lst = ovf_rows[c].get((h, t), [])
                lo = int(off[h, t])
                obase = (base[h] + nident) * PT
                for p in range(PT):
                    r = ch * PT + p - lo  # row index within the cell
                    if 0 <= r < len(lst):
                        tl, m1, m2 = lst[r]
                        idx_flat[obase + ch * PT + p] = tl
                        d1[p, j] = m1
                        if m2 >= 0:
                            d2[p, j] = m2
        smat = ((d1[:, :, None] == mcols).astype(np.int8)
                + (d2[:, :, None] == mcols).astype(np.int8)).astype(NPF8)
        per_core.append(dict(
            idx=_wrap_idxs(idx_flat),
            smat=np.ascontiguousarray(smat),
        ))

    # per-(strip, half) overflow chunk ranges
    strip_ovf = []
    for (s0, s1) in STRIPS:
        t_last = s1 - 1
        strip_ovf.append(tuple(
            (int(off[h, s0]) // PT,
             int(off[h, t_last] + rows_ext[h, t_last]) // PT)
            for h in range(2)))

    meta = dict(nshard=nshard, nt=nt, H0=H0, SLAB=SLAB, npadn=npadn,
                nch_ovf=nch_ovf, nident=nident, base=base, ncols=ncols,
                ovf_jobs_by_tile=ovf_jobs_by_tile, njobs=njobs,
                strip_ovf=strip_ovf)
    return meta, per_core


def _zrowb(dz: int) -> int:
    """fp8 z-table row bytes, padded to the 256B dma_gather granule."""
    return ((dz + 255) // 256) * 256


# ----------------------------------------------------------------------------
# device program builder
# ----------------------------------------------------------------------------

def _build_program(meta, dims, fc_out):
    nshard, nt = meta["nshard"], meta["nt"]
    H0, SLAB, npadn = meta["H0"], meta["SLAB"], meta["npadn"]
    nch_ovf, nident, base = meta["nch_ovf"], meta["nident"], meta["base"]
    ncols, njobs = meta["ncols"], meta["njobs"]
    ovf_jobs_by_tile = meta["ovf_jobs_by_tile"]
    din0 = dims[0][0]
    NROWS = 2 * SLAB

    wdims = []
    for (di, do) in dims:
        wdims.append((di, do))   # sage
        wdims.append((do, do))   # gc
    wdims.append((dims[-1][1], fc_out))  # fc
    # layer 0 aggregates raw features (din0); layers 1..5 aggregate z_k
    agg_dz = [din0] + [wdims[k][1] for k in range(1, 6)]
    zrowb = [_zrowb(d) for d in agg_dz]

    rows = [PT] * nt
    rows[nt - 1] = nshard - (nt - 1) * PT

    nc = bacc.Bacc("TRN2", target_bir_lowering=False, debug=False,
                   num_devices=NCORES, num_swdge_queues=NQUEUES)

    # fp8 feature tables: layer 0 gathers features directly (W0 applied
    # after aggregation), so no z0 table and no layer-0 AllGather. fh0/fh1
    # are host-replicated images of what the AG would have produced;
    # feat_own mirrors the z_own row layout for the local term.
    fh_in = [nc.dram_tensor(f"fh{h}", [NCORES * SLAB, din0], F8,
                            kind="ExternalInput") for h in (0, 1)]
    fown_in = nc.dram_tensor("fown", [NROWS, din0], F8, kind="ExternalInput")
    idx_in = nc.dram_tensor("idx", [PT, ncols * 8], I16,
                            kind="ExternalInput")
    smat_in = nc.dram_tensor("smat", [PT, njobs, PT], F8,
                             kind="ExternalInput")
    ii_in = nc.dram_tensor("ii", [PT, 2, PT], F8, kind="ExternalInput")
    twoi_in = nc.dram_tensor("twoi", [PT, PT], F8, kind="ExternalInput")
    ident_in = nc.dram_tensor("ident", [PT, PT], BF16, kind="ExternalInput")
    zscale_in = nc.dram_tensor("zscale", [PT, 6, nt], F32,
                               kind="ExternalInput")
    escale_in = nc.dram_tensor("escale", [PT, 6, nt], F32,
                               kind="ExternalInput")
    w_in = []
    for k, (di, do) in enumerate(wdims):
        w_in.append(nc.dram_tensor(f"w{k}", [PT, di // PT, do], BF16,
                                   kind="ExternalInput"))
    out = nc.dram_tensor("out", [nshard, fc_out], F32, kind="ExternalOutput")

    with tile.TileContext(nc) as tcx:
        with (
            tcx.tile_pool(name="const", bufs=1) as constp,
            tcx.tile_pool(name="g", bufs=12) as gp,
            tcx.tile_pool(name="zt", bufs=3) as ztp,
            tcx.tile_pool(name="pa", bufs=2) as pap,
            tcx.tile_pool(name="pf", bufs=2) as pfp,
            tcx.tile_pool(name="u", bufs=2) as up,
            tcx.tile_pool(name="h", bufs=3) as hp,
            tcx.tile_pool(name="ht", bufs=2) as htp,
            tcx.tile_pool(name="zo", bufs=3) as zop,
            tcx.tile_pool(name="z0", bufs=1) as z0p,
            tcx.tile_pool(name="aggps", bufs=3, space="PSUM") as aggpsp,
            tcx.tile_pool(name="trps", bufs=2, space="PSUM") as trpsp,
            tcx.tile_pool(name="mmps", bufs=2, space="PSUM") as mmpsp,
            tcx.tile_pool(name="dram", bufs=1, space="DRAM") as dramp,
        ):
            # tiny dummy AllGather issued first: absorbs first-collective
            # warmup / core-launch skew while stage B0 runs
            warm_in = dramp.tile([16, 256], F8, name="warm_in")
            warm_out = dramp.tile([NCORES * 16, 256], F8, name="warm_out",
                                  addr_space="Shared")
            nc.gpsimd.collective_compute(
                "AllGather", mybir.AluOpType.bypass,
                replica_groups=[list(range(NCORES))],
                ins=[warm_in[:].opt()], outs=[warm_out[:].opt()])

            # persistent constants (smat pinned for all 6 layers)
            idx_sb = constp.tile([PT, ncols * 8], I16, name="idx_sb")
            nc.sync.dma_start(idx_sb[:], idx_in[:])
            smat_sb = constp.tile([PT, njobs, PT], F8, name="smat_sb")
            nc.sync.dma_start(smat_sb[:], smat_in[:])
            ii_sb = constp.tile([PT, 2, PT], F8, name="ii_sb")
            nc.sync.dma_start(ii_sb[:], ii_in[:])
            twoi_sb = constp.tile([PT, PT], F8, name="twoi_sb")
            nc.sync.dma_start(twoi_sb[:], twoi_in[:])
            ident_sb = constp.tile([PT, PT], BF16, name="ident_sb")
            nc.sync.dma_start(ident_sb[:], ident_in[:])
            zscale_sb = constp.tile([PT, 6, nt], F32, name="zscale_sb")
            nc.sync.dma_start(zscale_sb[:], zscale_in[:])
            escale_sb = constp.tile([PT, 6, nt], F32, name="escale_sb")
            nc.sync.dma_start(escale_sb[:], escale_in[:])
            w_sb = []
            for k, (di, do) in enumerate(wdims):
                w = constp.tile([PT, di // PT, do], BF16, name=f"w{k}_sb")
                nc.sync.dma_start(w[:], w_in[k][:])
                w_sb.append(w)

            # internal DRAM: z shards + gathered half tables (k=1..5 only;
            # layer 0 reads the fh/fown inputs). Row 0 of each z_own is the
            # half-0 zero row; written once here.
            z_own, z_half = [None], [None]
            zseed = z0p.tile([PT, max(zrowb)], F8, name="zseed")
            nc.vector.memset(zseed[:], 0.0)
            for a in range(1, 6):
                zb = zrowb[a]
                z_own.append(dramp.tile([NROWS, zb], F8, name=f"zown{a}"))
                z_half.append([
                    dramp.tile([NCORES * SLAB, zb], F8,
                               name=f"zhalf{a}_{h}", addr_space="Shared")
                    for h in (0, 1)])
                nc.sync.dma_start(z_own[a][0:1, :], zseed[0:1, :zb])

            def run_ag(a, h):
                nc.gpsimd.collective_compute(
                    "AllGather", mybir.AluOpType.bypass,
                    replica_groups=[list(range(NCORES))],
                    ins=[z_own[a][h * SLAB:(h + 1) * SLAB, :].opt()],
                    outs=[z_half[a][h][:].opt()],
                )

            def stage_b(t, h_tile, k, dest, dest_dtype):
                """h_tile [128, Din_k] bf16 -> dest rows of tile t."""
                di, do = wdims[k]
                kg = di // PT
                ht = htp.tile([PT, kg, PT], BF16, tag="ht")
                for g in range(kg):
                    trp = trpsp.tile([PT, PT], BF16, tag="tr")
                    nc.tensor.transpose(trp[:], h_tile[:, g * PT:(g + 1) * PT],
                                        ident_sb[:])
                    nc.vector.tensor_copy(ht[:, g, :], trp[:])
                zp = mmpsp.tile([PT, do], F32, tag="mm")
                for g in range(kg):
                    nc.tensor.matmul(zp[:], ht[:, g, :], w_sb[k][:, g, :],
                                     start=(g == 0), stop=(g == kg - 1))
                zo = zop.tile([PT, do], dest_dtype, tag="zo")
                if dest_dtype == F32:  # fc output: unpadded dest
                    nc.vector.tensor_copy(zo[:], zp[:])
                    nc.sync.dma_start(dest[t * PT:t * PT + rows[t], :],
                                      zo[:rows[t], :])
                else:  # z table: fp8 cast with per-node scale; +1 row offset
                    nc.scalar.activation(zo[:], zp[:],
                                         mybir.ActivationFunctionType.Copy,
                                         scale=zscale_sb[:, k, t:t + 1])
                    nc.sync.dma_start(
                        dest[1 + t * PT:1 + (t + 1) * PT, :do], zo[:])

            ag_t = (H0 + PT - 1) // PT - 1  # slab 0 complete after this tile

            # ---- 6 aggregation layers + following stage B ----
            call_id = 0
            for a in range(6):
                dz = agg_dz[a]
                zb = zrowb[a]
                sage = (a % 2 == 0)
                k_next = a + 1
                is_fc = (k_next == 6)

                chunkmap = {}   # (kind, h, chunk) -> (g tile, slot)
                cursor = {}     # (kind, si, h) -> next chunk to emit
                ztab = fh_in if a == 0 else z_half[a]
                zown_src = fown_in if a == 0 else z_own[a]

                def emit_calls(kind, si, h, upto_chunk):
                    """kind 'i': ident chunks (strip range [s0*Q, s1*Q));
                    kind 'o': overflow chunks (meta strip ranges)."""
                    nonlocal call_id
                    if kind == "i":
                        s0, s1 = STRIPS[si]
                        c_lo, c_hi = s0 * QLVL, s1 * QLVL
                        col0 = base[h]
                    else:
                        c_lo, c_hi = meta["strip_ovf"][si][h]
                        col0 = base[h] + nident
                    cur = cursor.setdefault((kind, si, h), c_lo)
                    while cur <= upto_chunk and cur < c_hi:
                        cw = min(MAXC, c_hi - cur)
                        g = gp.tile([PT, MAXC, zb], F8, tag="g")
                        col = col0 + cur
                        nc.gpsimd.dma_gather(
                            g[:, :cw, :], ztab[h][:],
                            idx_sb[:, col * 8:(col + cw) * 8],
                            cw * PT, cw * PT, zb,
                            single_packet=SINGLE_PACKET,
                            queue_num=call_id % NQUEUES)
                        for k in range(cw):
                            chunkmap[(kind, h, cur + k)] = (g, k)
                        cur += cw
                        cursor[(kind, si, h)] = cur
                        call_id += 1

                def agg_matmuls(aggp, jt, first_starts):
                    """jt: list of (kind, h, c, sji); sji = smat job index
                    for 'o' jobs (ignored for 'i'). Consecutive same-call
                    chunk pairs run as one DoubleRow matmul."""
                    nj = len(jt)
                    i = 0
                    first = True
                    while i < nj:
                        kind, h, c, sji = jt[i]
                        gt, slot = chunkmap[(kind, h, c)]
                        w = 1
                        if i + 1 < nj:
                            k2, h2, c2, sji2 = jt[i + 1]
                            if (k2 == kind and h2 == h and c2 == c + 1
                                    and (kind == "i" or sji2 == sji + 1)):
                                gt2, slot2 = chunkmap[(kind, h2, c2)]
                                if gt2 is gt and slot2 == slot + 1:
                                    w = 2
                        start = first and first_starts
                        stop = (i + w == nj)
                        if w == 2:
                            lhs2 = (ii_sb[:, 0:2, :] if kind == "i"
                                    else smat_sb[:, sji:sji + 2, :])
                            nc.tensor.matmul(
                                aggp[:], lhs2, gt[:, slot:slot + 2, :dz],
                                start=start, stop=stop,
                                perf_mode=mybir.MatmulPerfMode.DoubleRow)
                        else:
                            lhs1 = (ii_sb[:, 0, :] if kind == "i"
                                    else smat_sb[:, sji, :])
                            nc.tensor.matmul(
                                aggp[:], lhs1, gt[:, slot, :dz],
                                start=start, stop=stop)
                        first = False
                        i += w

                def tile_jobs(t, want_h):
                    """Job list for tile t restricted to half want_h (or both
                    if None): ident jobs then overflow jobs."""
                    jt = []
                    for h in (0, 1):
                        if want_h is not None and h != want_h:
                            continue
                        for c in range(t * QLVL, (t + 1) * QLVL):
                            jt.append(("i", h, c, -1))
                    for (h, c, j) in ovf_jobs_by_tile[t]:
                        if want_h is not None and h != want_h:
                            continue
                        jt.append(("o", h, c, j))
                    return jt

                def emit_for(t, si, want_h):
                    for h in (0, 1):
                        if want_h is not None and h != want_h:
                            continue
                        emit_calls("i", si, h, (t + 1) * QLVL - 1)
                        oc = [c for (hh, c, _) in ovf_jobs_by_tile[t]
                              if hh == h]
                        if oc:
                            emit_calls("o", si, h, max(oc))

                def local_mms(t, aggp, zt):
                    lhs = twoi_sb[:] if sage else ii_sb[:, 0, :]
                    nc.tensor.matmul(aggp[:], lhs, zt[:, :dz],
                                     start=True, stop=False)

                def epilogue(t, src_ap):
                    """relu(scale * src) -> stage_b -> (maybe) first AG."""
                    if a == 0:
                        # X = agg/(den*alpha_f); h1 = relu(X @ W0^T) -> z1
                        di, do = wdims[0]
                        X = hp.tile([PT, di], BF16, tag="h")
                        nc.scalar.activation(X[:], src_ap,
                                             mybir.ActivationFunctionType.Copy,
                                             scale=escale_sb[:, 0, t:t + 1])
                        kg = di // PT
                        ht = htp.tile([PT, kg, PT], BF16, tag="ht")
                        for g in range(kg):
                            trp = trpsp.tile([PT, PT], BF16, tag="tr")
                            nc.tensor.transpose(trp[:],
                                                X[:, g * PT:(g + 1) * PT],
                                                ident_sb[:])
                            nc.vector.tensor_copy(ht[:, g, :], trp[:])
                        zp = mmpsp.tile([PT, do], F32, tag="mm")
                        for g in range(kg):
                            nc.tensor.matmul(zp[:], ht[:, g, :],
                                             w_sb[0][:, g, :],
                                             start=(g == 0),
                                             stop=(g == kg - 1))
                        hx = hp.tile([PT, do], BF16, tag="h")
                        nc.scalar.activation(hx[:], zp[:],
                                             mybir.ActivationFunctionType.Relu)
                        stage_b(t, hx, 1, z_own[1], F8)
                        if t == ag_t:
                            run_ag(1, 0)
                        return
                    hx = hp.tile([PT, dz], BF16, tag="h")
                    nc.scalar.activation(hx[:], src_ap,
                                         mybir.ActivationFunctionType.Relu,
                                         scale=escale_sb[:, a, t:t + 1])
                    if is_fc:
                        stage_b(t, hx, 6, out, F32)
                    else:
                        stage_b(t, hx, k_next, z_own[k_next], F8)
                        if t == ag_t:
                            run_ag(k_next, 0)

                def single_pass(t, si):
                    emit_for(t, si, None)
                    zt = ztp.tile([PT, zb], F8, tag="zt")
                    nc.sync.dma_start(
                        zt[:], zown_src[1 + t * PT:1 + (t + 1) * PT, :])
                    aggp = aggpsp.tile([PT, dz], F32, tag="agg")
                    local_mms(t, aggp, zt)
                    agg_matmuls(aggp, tile_jobs(t, None), False)
                    epilogue(t, aggp[:])

                # Layer 0's tables are inputs (no AllGather to hide):
                # single-pass everywhere. Later layers run strip 0 in two
                # phases (table-0 only, then table-1) to absorb the
                # end-of-previous-layer AllGather of table 1.
                if a == 0:
                    for si in range(len(STRIPS)):
                        for t in range(*STRIPS[si]):
                            single_pass(t, si)
                else:
                    tiles0 = list(range(*STRIPS[0]))
                    pa = pap.tile([PT, len(tiles0), dz], BF16, tag="pa")
                    for i, t in enumerate(tiles0):
                        emit_for(t, 0, 0)
                        zt = ztp.tile([PT, zb], F8, tag="zt")
                        nc.sync.dma_start(
                            zt[:], zown_src[1 + t * PT:1 + (t + 1) * PT, :])
                        aggp = aggpsp.tile([PT, dz], F32, tag="agg")
                        local_mms(t, aggp, zt)
                        agg_matmuls(aggp, tile_jobs(t, 0), False)
                        nc.scalar.activation(pa[:, i, :], aggp[:],
                                             mybir.ActivationFunctionType.Copy)
                    for i, t in enumerate(tiles0):
                        emit_for(t, 0, 1)
                        aggp = aggpsp.tile([PT, dz], F32, tag="agg")
                        agg_matmuls(aggp, tile_jobs(t, 1), True)
                        pf = pfp.tile([PT, dz], F32, tag="pf")
                        nc.scalar.activation(pf[:], pa[:, i, :],
                                             mybir.ActivationFunctionType.Copy)
                        u = up.tile([PT, dz], F32, tag="u")
                        nc.vector.tensor_tensor(u[:], aggp[:], pf[:],
                                                mybir.AluOpType.add)
                        epilogue(t, u[:])
                    for si in range(1, len(STRIPS)):
                        for t in range(*STRIPS[si]):
                            single_pass(t, si)
                if not is_fc:
                    run_ag(k_next, 1)

    t0 = time.time()
    nc.compile()
    print(f"[kernel] bacc compile: {time.time() - t0:.1f}s", file=sys.stderr)
    return nc


# ----------------------------------------------------------------------------
# public entry
# ----------------------------------------------------------------------------

_CACHE = {}


def _build_in_maps(features, src, dst, sage_ws, gc_ws, fc_w, meta, per_core):
    n_nodes, din0 = features.shape
    nshard, nt = meta["nshard"], meta["nt"]

    e_ones = np.ones(len(src), np.float64)
    in_deg = np.bincount(dst, weights=e_ones, minlength=n_nodes)
    out_deg = np.bincount(src, weights=e_ones, minlength=n_nodes)
    inv_den = (1.0 / (in_deg + 1.0)).astype(np.float32)
    inv_dst = (np.where(in_deg > 0, in_deg, 1.0) ** -0.5).astype(np.float32)
    inv_src = (np.where(out_deg > 0, out_deg, 1.0) ** -0.5).astype(np.float32)

    def shard_scal(v, c):
        s = np.ones(nt * PT, np.float32)
        s[:nshard] = v[c * nshard:(c + 1) * nshard]
        return np.ascontiguousarray(s.reshape(nt, PT).T)

    worder = []
    for s in range(len(sage_ws)):
        worder.append(sage_ws[s])
        worder.append(gc_ws[s])
    worder.append(fc_w)
    w_arrs = []
    for w in worder:
        do, di = w.shape
        wt = np.ascontiguousarray(
            w.T.astype(np.float32).reshape(di // PT, PT, do)
            .transpose(1, 0, 2)).astype(ml_dtypes.bfloat16)
        w_arrs.append(wt)

    ident = np.eye(PT, dtype=np.float32).astype(ml_dtypes.bfloat16)
    eye = np.eye(PT, dtype=np.float32)
    ii = np.ascontiguousarray(
        np.stack([eye, eye], axis=1).astype(NPF8))       # [PT, 2, PT]
    twoi = np.ascontiguousarray((2.0 * eye).astype(NPF8))

    # fp8 feature tables (alpha-scaled): fh0/fh1 replicate every core's
    # shard-half (z_half layout, incl zero rows); fown mirrors z_own rows.
    H0, SLAB = meta["H0"], meta["SLAB"]
    NROWS = 2 * SLAB
    f8 = (features.astype(np.float32) * ALPHA[0]).astype(NPF8)
    fh = [np.zeros((NCORES * SLAB, din0), NPF8) for _ in range(2)]
    fown_all = []
    for c in range(NCORES):
        sh = f8[c * nshard:(c + 1) * nshard]
        fh[0][c * SLAB + 1:c * SLAB + 1 + H0] = sh[:H0]
        fh[1][c * SLAB:c * SLAB + nshard - H0] = sh[H0:]
        fo = np.zeros((NROWS, din0), NPF8)
        fo[1:1 + nshard] = sh
        fown_all.append(fo)
    fh = [np.ascontiguousarray(x) for x in fh]

    # pad-node rows (t = nt-1, partitions >= rows[-1]) must produce z = 0:
    # zero their epilogue scale so hx and every later z stay 0 (gathered
    # zero rows and the AG slab-1 zero row depend on it)
    rows_last = nshard - (nt - 1) * PT
    in_maps = []
    for c in range(NCORES):
        zscale = np.empty((PT, 6, nt), np.float32)
        escale = np.empty((PT, 6, nt), np.float32)
        isrc = shard_scal(inv_src, c)
        iden = shard_scal(inv_den, c)
        idst = shard_scal(inv_dst, c)
        for k in range(6):
            zscale[:, k, :] = ALPHA[k] * (isrc if k % 2 == 1 else 1.0)
            escale[:, k, :] = (iden if k % 2 == 0 else idst) / ALPHA[k]
        escale[rows_last:, :, nt - 1] = 0.0
        im = dict(
            fh0=fh[0],
            fh1=fh[1],
            fown=fown_all[c],
            idx=per_core[c]["idx"],
            smat=per_core[c]["smat"],
            ii=ii,
            twoi=twoi,
            ident=ident,
            zscale=zscale,
            escale=escale,
        )
        for k, w in enumerate(w_arrs):
            im[f"w{k}"] = w
        in_maps.append(im)
    return in_maps


def _run(features, src, dst, sage_ws, sage_bs, gc_ws, gc_bs, fc_w, fc_b):
    n_nodes, din0 = features.shape
    dims = [(w.shape[1], w.shape[0]) for w in sage_ws]
    fc_out = fc_w.shape[0]

    key = hashlib.sha1(
        np.asarray(src).tobytes() + np.asarray(dst).tobytes()
        + str((n_nodes, din0, dims, fc_out)).encode()
    ).hexdigest()
    if key in _CACHE:
        nc, meta, per_core = _CACHE[key]
    else:
        meta, per_core = _prep_graph(src, dst, n_nodes)
        nc = _build_program(meta, dims, fc_out)
        _CACHE[key] = (nc, meta, per_core)

    biases = list(sage_bs) + list(gc_bs) + [fc_b]
    if any(np.any(np.asarray(b) != 0) for b in biases):
        raise NotImplementedError("nonzero biases not supported")

    in_maps = _build_in_maps(features, src, dst, sage_ws, gc_ws, fc_w,
                             meta, per_core)

    trace = bool(int(os.environ.get("GNN_TRACE", "0")))
    res = bass_utils.run_bass_kernel_spmd(
        nc, in_maps, core_ids=list(range(NCORES)), trace=trace)
    out = np.concatenate([res.results[c]["out"] for c in range(NCORES)],
                         axis=0).astype(np.float32)
    if trace:
        print(f"[kernel] exec_time_ns: {res.exec_time_ns}", file=sys.stderr)
        _CACHE["last_exec_time_ns"] = res.exec_time_ns
        _CACHE["last_profile"] = res.profile_json
    return out


def kernel(features, src, dst,
           sage_w0, sage_b0, gc_w0, gc_b0,
           sage_w1, sage_b1, gc_w1, gc_b1,
           sage_w2, sage_b2, gc_w2, gc_b2,
           fc_w, fc_b):
    features = np.asarray(features, np.float32)
    src = np.asarray(src, np.int64)
    dst = np.asarray(dst, np.int64)
    return _run(
        features, src, dst,
        [np.asarray(sage_w0), np.asarray(sage_w1), np.asarray(sage_w2)],
        [np.asarray(sage_b0), np.asarray(sage_b1), np.asarray(sage_b2)],
        [np.asarray(gc_w0), np.asarray(gc_w1), np.asarray(gc_w2)],
        [np.asarray(gc_b0), np.asarray(gc_b1), np.asarray(gc_b2)],
        np.asarray(fc_w), np.asarray(fc_b),
    )
